# revision 12
# baseline (speedup 1.0000x reference)
"""Block-sparse 3-layer MLP on 8 Trainium2 NeuronCores, via 1-level Strassen.

Reference computation (fp32):
    h1 = relu(x @ (W1*expand(mask1)).T + b1)       x:[B,2048] W1:[4096,2048]
    h2 = relu(h1 @ (W2*expand(mask2)).T + b2)      W2:[4096,4096]
    out = h2 @ Wo.T + bo                           Wo:[1024,4096] -> [B,1024]

Strategy: data-parallel over the batch (B=8192 -> bc=1024 per core), no
collectives, feature-major activations [features, batch].  The masks make the
weights block-sparse (32x32 tiles, i.i.d. 0.5 density) but i.i.d. 32-granular
sparsity cannot beat dense on a 128x128 PE (any 4-row/4-col-block packing is
~94% dense by the union bound) and fp8 fails the 2e-2 gate (measured 6.2e-2
one-pass, 4.4e-2 with a 2-term split).  Dense bf16 streams at 216ns per
[128x128]x[128x512] matmul and the dense baseline already ran at 95.4% PE
occupancy, so the remaining lever is cutting PE work itself:

Each layer h = W.x runs 1-level Strassen on the 2x2 block split of W
([m/2,k/2] quadrants) and of the feature-major activation ([k/2, 512]
quadrants; the 1024 batch splits into two 512 halves):
    M1=(A11+A22)(B11+B22) M2=(A21+A22)B11 M3=A11(B12-B22) M4=A22(B21-B11)
    M5=(A11+A12)B22 M6=(A21-A11)(B11+B12) M7=(A12-A22)(B21+B22)
    C11=M1+M4-M5+M7 C12=M3+M5 C21=M2+M4 C22=M1-M2+M3+M6
7 half-size products instead of 8: 3136 matmuls/core vs 3584 dense (PE floor
677us vs 773us).  Measured numeric cost: 7.5e-3 rel err vs 3.9e-3 dense bf16.

- A-side combos are free (host precomputes 7 bf16 lhsT panel sets per layer;
  1.75x weight HBM bytes ~ 103MB/core ~ 150GB/s sustained, well within ring
  fanout).  x-side B combos are host-computed too (x is an input).
- h1's B-combos for L2 are built on-device per row tile (5 bf16
  tensor_tensor adds on gpsimd, overlapped with products); h1's C12/C21
  quadrant tiles are freed right after (rotating pool) since only C11/C22
  are consumed raw (M2/M5).  h2's combos cannot coexist with h1's in SBUF,
  so they are deferred to L3 start and built just-in-time under L3 row 0's
  M2/M5 products (which read raw h2 quadrants), split across gpsimd+vector.
- C-combines run on the vector engine as scalar_tensor_tensor chains
  (out=(in0 op0 scalar) op1 in1), each reading exactly one PSUM operand (ISA
  limit) plus one SBUF f32 partial.  M1/M4/M5 are evicted to SBUF by the
  scalar engine (activation IDENT); bias rides the STT scalar slot; relu is
  a vector tensor_scalar_max into the resident bf16 h tiles.  In L3 the
  final STT of each quadrant writes the bf16 output tile directly.
- PSUM: every product accumulates over k/2 into its own [128,512] f32 tile
  (exactly one PSUM bank); peak ~4 live banks per row, one 8-deep pool
  rotates across rows and layers without PE stalls.
- lhsT panels stream per-product on the three DMA rings (sync/scalar/
  gpsimd), 1-2 products ahead; row-0 panels and the first x tiles are split
  finer so the first matmul issues ~2us in.
"""

import sys

sys.path.insert(0, "/opt/trn_rl_repo")

import numpy as np

from concourse import bacc, mybir, tile
from concourse.bass_utils import run_bass_kernel_spmd

F32 = mybir.dt.float32
BF16 = mybir.dt.bfloat16
IDENT = mybir.ActivationFunctionType.Identity
ADD = mybir.AluOpType.add
SUB = mybir.AluOpType.subtract
MULT = mybir.AluOpType.mult

N_CORES = 8
TILE = 32
P = 128
NH = 512  # half-batch strip = one psum bank

# per-row product order: raw-B products first (startup / lazy-combo cover),
# early-evicted M1 next, then combine-dependency order
PRODUCT_ORDER = [2, 5, 1, 4, 6, 3, 7]
PRODUCT_ORDER_LAST = [2, 5, 1, 4, 6, 7, 3]
PROD_B = {1: "g1", 2: "b11", 3: "g3", 4: "g4", 5: "b22", 6: "g6", 7: "g7"}


def _build(nc, d_in, d_h, d_out, bc):
    kt1 = d_in // 2 // P   # 8  k-tiles per L1 product
    kt2 = d_h // 2 // P    # 16 k-tiles per L2/L3 product
    rt12 = d_h // 2 // P   # 16 row tiles per L1/L2 quadrant
    rt3 = d_out // 2 // P  # 4  row tiles per L3 quadrant

    xb11_d = nc.dram_tensor("xb11", [kt1, P, NH], BF16, kind="ExternalInput")
    xb22_d = nc.dram_tensor("xb22", [2, P, 4 * NH], BF16, kind="ExternalInput")
    xc_d = {
        j: nc.dram_tensor(f"xc{j}", [P, kt1 * NH], BF16, kind="ExternalInput")
        for j in (1, 3, 4, 6, 7)
    }
    w1_d = {j: nc.dram_tensor(f"w1_{j}", [rt12, P, kt1 * P], BF16,
                              kind="ExternalInput") for j in range(1, 8)}
    w2_d = {j: nc.dram_tensor(f"w2_{j}", [rt12, P, kt2 * P], BF16,
                              kind="ExternalInput") for j in range(1, 8)}
    wo_d = {j: nc.dram_tensor(f"wo_{j}", [rt3, P, kt2 * P], BF16,
                              kind="ExternalInput") for j in range(1, 8)}
    b1t_d = nc.dram_tensor("b1t", [P, rt12], F32, kind="ExternalInput")
    b1b_d = nc.dram_tensor("b1b", [P, rt12], F32, kind="ExternalInput")
    b2t_d = nc.dram_tensor("b2t", [P, rt12], F32, kind="ExternalInput")
    b2b_d = nc.dram_tensor("b2b", [P, rt12], F32, kind="ExternalInput")
    bot_d = nc.dram_tensor("bot", [P, rt3], F32, kind="ExternalInput")
    bob_d = nc.dram_tensor("bob", [P, rt3], F32, kind="ExternalInput")
    out_d = nc.dram_tensor("out", [4 * rt3, P, NH], BF16, kind="ExternalOutput")

    with tile.TileContext(nc) as tc:
        with (
            tc.tile_pool(name="bias", bufs=1) as bias_pool,
            tc.tile_pool(name="ev", bufs=3) as ev_pool,
            tc.tile_pool(name="ch", bufs=6) as ch_pool,
            tc.tile_pool(name="ps", bufs=8, space="PSUM") as ps_pool,
        ):
            b1t = bias_pool.tile([P, rt12], F32, tag="b1t")
            b1b = bias_pool.tile([P, rt12], F32, tag="b1b")
            b2t = bias_pool.tile([P, rt12], F32, tag="b2t")
            b2b = bias_pool.tile([P, rt12], F32, tag="b2b")
            bot = bias_pool.tile([P, rt3], F32, tag="bot")
            bob = bias_pool.tile([P, rt3], F32, tag="bob")
            rings = [nc.sync, nc.scalar, nc.gpsimd]

            def emit_layer(lay, rows, kts, rhs, bt_sb, bb_sb, panel_pool,
                           panel_dram, panel_w, lookahead, prefetch_hook,
                           row_hook, out_cb, interleave0=False):
                panels = {}
                pf = {"next": 0}
                def row_order(r):
                    return PRODUCT_ORDER_LAST if r == rows - 1 else PRODUCT_ORDER

                order = [(r, jp) for r in range(rows) for jp in range(7)]

                def issue_panel(idx, split):
                    r, jp = order[idx]
                    j = row_order(r)[jp]
                    t = panel_pool.tile([P, panel_w], BF16, tag=f"pan{lay}")
                    if split == 1:
                        rings[idx % 3].dma_start(out=t[:], in_=panel_dram[j][r])
                    else:
                        w = panel_w // split
                        for s in range(split):
                            rings[(idx + s) % 3].dma_start(
                                out=t[:, s * w:(s + 1) * w],
                                in_=panel_dram[j][r][:, s * w:(s + 1) * w],
                            )
                    panels[(r, j)] = t

                def pump(upto):
                    while pf["next"] <= min(upto, len(order) - 1):
                        issue_panel(
                            pf["next"],
                            4 if (lay == 1 and pf["next"] < 2) else 1,
                        )
                        pf["next"] += 1

                pump(0)
                for r in range(rows):
                    ps = {}
                    e = {}
                    bt = bt_sb[:, r:r + 1]
                    bb = bb_sb[:, r:r + 1]
                    po = row_order(r)
                    start_jp = 0
                    if r == 0 and interleave0:
                        # interleave M2/M5 k-tile-by-k-tile: their kt15 rhs
                        # tiles are the previous layer's last-row outputs;
                        # 30 matmuls of cover run before either is needed
                        pump(1 + lookahead)
                        prefetch_hook(0, 0)
                        prefetch_hook(0, 1)
                        pans = [panels.pop((0, po[0])), panels.pop((0, po[1]))]
                        for ji, j in enumerate(po[:2]):
                            ps[j] = ps_pool.tile([P, NH], F32, name=f"psi{j}",
                                                 tag="ps")
                        for kt in range(kts):
                            for ji, j in enumerate(po[:2]):
                                nc.tensor.matmul(
                                    ps[j][:],
                                    pans[ji][:, kt * P:(kt + 1) * P],
                                    rhs(j, kt),
                                    start=(kt == 0),
                                    stop=(kt == kts - 1),
                                )
                        e[5] = ev_pool.tile([P, NH], F32, name="e5", tag="ev")
                        nc.scalar.activation(e[5][:], ps[5][:], IDENT)
                        start_jp = 2
                    for jp, j in enumerate(po):
                        if jp < start_jp:
                            continue
                        idx = r * 7 + jp
                        pump(idx + lookahead)
                        prefetch_hook(r, jp)
                        pan = panels.pop((r, j))
                        pst = ps_pool.tile([P, NH], F32, tag="ps")
                        for kt in range(kts):
                            nc.tensor.matmul(
                                pst[:],
                                pan[:, kt * P:(kt + 1) * P],
                                rhs(j, kt),
                                start=(kt == 0),
                                stop=(kt == kts - 1),
                            )
                        ps[j] = pst
                        # combine DAG, emitted as operands become available
                        if j == 5:
                            e[5] = ev_pool.tile([P, NH], F32, name="e5", tag="ev")
                            nc.scalar.activation(e[5][:], ps[5][:], IDENT)
                        elif j == 1:
                            e[1] = ev_pool.tile([P, NH], F32, name="e1", tag="ev")
                            nc.scalar.activation(e[1][:], ps[1][:], IDENT)
                        elif j == 4:
                            e[4] = ev_pool.tile([P, NH], F32, name="e4", tag="ev")
                            nc.scalar.activation(e[4][:], ps[4][:], IDENT)
                            # C21 = M2 + M4 + bb
                            s21 = ch_pool.tile([P, NH], F32, tag="ch")
                            nc.vector.scalar_tensor_tensor(
                                s21[:], ps[2][:], bb, e[4][:], ADD, ADD)
                            out_cb("c21", r, s21)
                        elif j == 6:
                            # C22 = M1 - M2 + M3 + M6 + bb
                            d1 = ch_pool.tile([P, NH], F32, tag="ch")
                            nc.vector.scalar_tensor_tensor(
                                d1[:], ps[6][:], bb, e[1][:], ADD, ADD)
                            d2 = ch_pool.tile([P, NH], F32, tag="ch")
                            nc.vector.scalar_tensor_tensor(
                                d2[:], ps[2][:], -1.0, d1[:], MULT, ADD)
                            e["d2"] = d2
                        elif j == 3:
                            # C12 = M3 + M5 + bt
                            s12 = ch_pool.tile([P, NH], F32, tag="ch")
                            nc.vector.scalar_tensor_tensor(
                                s12[:], ps[3][:], bt, e[5][:], ADD, ADD)
                            out_cb("c12", r, s12)
                            d3 = ch_pool.tile([P, NH], F32, tag="ch")
                            nc.vector.scalar_tensor_tensor(
                                d3[:], ps[3][:], 1.0, e["d2"][:], MULT, ADD)
                            out_cb("c22", r, d3)
                        elif j == 7:
                            # C11 = M1 + M4 - M5 + M7 + bt
                            c1 = ch_pool.tile([P, NH], F32, tag="ch")
                            nc.vector.scalar_tensor_tensor(
                                c1[:], ps[7][:], bt, e[1][:], ADD, ADD)
                            c2 = ch_pool.tile([P, NH], F32, tag="ch")
                            nc.vector.scalar_tensor_tensor(
                                c2[:], e[4][:], 1.0, c1[:], MULT, ADD)
                            c3 = ch_pool.tile([P, NH], F32, tag="ch")
                            nc.vector.scalar_tensor_tensor(
                                c3[:], e[5][:], -1.0, c2[:], MULT, ADD)
                            out_cb("c11", r, c3)
                    row_hook(r)

            # =================== Layer 1 ===================
            es_h1c11 = tc.tile_pool(name="h1c11", bufs=1, side="left")
            h1c11_pool = es_h1c11.__enter__()
            es_h1c22 = tc.tile_pool(name="h1c22", bufs=1, side="left")
            h1c22_pool = es_h1c22.__enter__()
            es_g = tc.tile_pool(name="g", bufs=1, side="left")
            g_pool = es_g.__enter__()
            h1c11, h1c22 = [], []
            g = {j: [] for j in (1, 3, 4, 6, 7)}

            es_x = tc.tile_pool(name="x", bufs=1, side="left")
            x_pool = es_x.__enter__()
            es_w1 = tc.tile_pool(name="w1", bufs=4, side="left")
            w1_pool = es_w1.__enter__()
            es_h1rot = tc.tile_pool(name="h1rot", bufs=3, side="left")
            h1rot_pool = es_h1rot.__enter__()

            xb11 = [x_pool.tile([P, NH], BF16, name=f"xb11_{k}", tag=f"xb11_{k}")
                    for k in range(kt1)]
            xb22 = [x_pool.tile([P, 4 * NH], BF16, name=f"xb22_{k}", tag=f"xb22_{k}")
                    for k in range(2)]
            xc = {j: x_pool.tile([P, kt1 * NH], BF16, name=f"xc{j}", tag=f"xc{j}")
                  for j in (1, 3, 4, 6, 7)}
            nc.sync.dma_start(out=xb11[0][:], in_=xb11_d[0])
            nc.scalar.dma_start(out=xb11[1][:], in_=xb11_d[1])
            nc.gpsimd.dma_start(out=xb11[2][:], in_=xb11_d[2])

            def half(eng, t, dsrc, s):
                hw = t.shape[1] // 2
                eng.dma_start(out=t[:, s * hw:(s + 1) * hw],
                              in_=dsrc[:, s * hw:(s + 1) * hw])

            def l1_prefetch(r, jp):
                # need order: xb22 (jp1), xc1 (jp2), xc4 (jp3), xc6, xc3, xc7;
                # halves spread over the three rings, all in flight by jp3
                if r == 0 and jp == 0:
                    for k in range(3, kt1):
                        rings[k % 3].dma_start(out=xb11[k][:], in_=xb11_d[k])
                    half(nc.sync, xb22[0], xb22_d[0], 0)
                    half(nc.gpsimd, xb22[0], xb22_d[0], 1)
                    half(nc.scalar, xc[1], xc_d[1], 0)
                    half(nc.sync, xc[1], xc_d[1], 1)
                elif r == 0 and jp == 1:
                    half(nc.gpsimd, xc[4], xc_d[4], 0)
                    half(nc.scalar, xc[4], xc_d[4], 1)
                    half(nc.sync, xb22[1], xb22_d[1], 0)
                    half(nc.gpsimd, xb22[1], xb22_d[1], 1)
                elif r == 0 and jp == 2:
                    half(nc.scalar, xc[6], xc_d[6], 0)
                    half(nc.sync, xc[6], xc_d[6], 1)
                    half(nc.gpsimd, xc[3], xc_d[3], 0)
                    half(nc.scalar, xc[3], xc_d[3], 1)
                    nc.scalar.dma_start(out=b1t[:], in_=b1t_d[:])
                    nc.scalar.dma_start(out=b1b[:], in_=b1b_d[:])
                elif r == 0 and jp == 3:
                    half(nc.sync, xc[7], xc_d[7], 0)
                    half(nc.gpsimd, xc[7], xc_d[7], 1)
                elif r == 1 and jp == 0:
                    nc.scalar.dma_start(out=b2t[:], in_=b2t_d[:])
                    nc.scalar.dma_start(out=b2b[:], in_=b2b_d[:])

            def l1_rhs(j, kt):
                b = PROD_B[j]
                if b == "b11":
                    return xb11[kt][:]
                if b == "b22":
                    return xb22[kt // 4][:, (kt % 4) * NH:(kt % 4 + 1) * NH]
                return xc[j][:, kt * NH:(kt + 1) * NH]

            rowstate = {}

            def l1_out(quad, r, src):
                if quad == "c11":
                    t = h1c11_pool.tile([P, NH], BF16, name=f"h1c11_{r}",
                                        tag=f"h1c11_{r}")
                    h1c11.append(t)
                elif quad == "c22":
                    t = h1c22_pool.tile([P, NH], BF16, name=f"h1c22_{r}",
                                        tag=f"h1c22_{r}")
                    h1c22.append(t)
                else:
                    t = h1rot_pool.tile([P, NH], BF16, tag=f"h1rot_{quad}")
                    rowstate[quad] = t
                nc.vector.tensor_scalar_max(t[:], src[:], 0.0)

            def l1_row_hook(r):
                c12, c21 = rowstate["c12"], rowstate["c21"]
                c11, c22 = h1c11[r], h1c22[r]
                for j, (a, b, op) in (
                    (1, (c11, c22, ADD)), (3, (c12, c22, SUB)),
                    (4, (c21, c11, SUB)), (6, (c11, c12, ADD)),
                    (7, (c21, c22, ADD)),
                ):
                    t = g_pool.tile([P, NH], BF16, name=f"g{j}_{r}", tag=f"g{j}_{r}")
                    nc.vector.tensor_tensor(t[:], a[:], b[:], op)
                    g[j].append(t)

            emit_layer(1, rt12, kt1, l1_rhs, b1t, b1b, w1_pool, w1_d,
                       kt1 * P, 2, l1_prefetch, l1_row_hook, l1_out)

            es_h1rot.__exit__(None, None, None)
            es_w1.__exit__(None, None, None)
            es_x.__exit__(None, None, None)

            # =================== Layer 2 ===================
            es_h2c11 = tc.tile_pool(name="h2c11", bufs=1, side="right")
            h2c11_pool = es_h2c11.__enter__()
            es_h2c22 = tc.tile_pool(name="h2c22", bufs=1, side="right")
            h2c22_pool = es_h2c22.__enter__()
            es_h2c12 = tc.tile_pool(name="h2c12", bufs=1, side="right")
            h2c12_pool = es_h2c12.__enter__()
            es_h2c21 = tc.tile_pool(name="h2c21", bufs=1, side="right")
            h2c21_pool = es_h2c21.__enter__()
            es_w2 = tc.tile_pool(name="w2", bufs=3, side="left")
            w2_pool = es_w2.__enter__()
            h2 = {"c11": [], "c12": [], "c21": [], "c22": []}
            h2pools = {"c11": h2c11_pool, "c12": h2c12_pool,
                       "c21": h2c21_pool, "c22": h2c22_pool}

            def l2_rhs(j, kt):
                b = PROD_B[j]
                if b == "b11":
                    return h1c11[kt][:]
                if b == "b22":
                    return h1c22[kt][:]
                return g[j][kt][:]

            def l2_out(quad, r, src):
                t = h2pools[quad].tile([P, NH], BF16, name=f"h2{quad}_{r}",
                                       tag=f"h2{quad}_{r}")
                nc.vector.tensor_scalar_max(t[:], src[:], 0.0)
                h2[quad].append(t)

            emit_layer(2, rt12, kt2, l2_rhs, b2t, b2b, w2_pool, w2_d,
                       kt2 * P, 1, lambda r, jp: None, lambda r: None, l2_out,
                       interleave0=True)

            es_w2.__exit__(None, None, None)
            es_g.__exit__(None, None, None)
            es_h1c22.__exit__(None, None, None)
            es_h1c11.__exit__(None, None, None)

            # =================== Layer 3 ===================
            # Phase A: the 8 raw-B products (M2/M5 of all 4 rows) run first,
            # each evicted to SBUF f32 by the scalar engine on completion;
            # all 80 h2 B-combos build on the vector engine under that
            # ~28us of PE cover (gpsimd tensor_tensor measured ~1.4us/tile,
            # 4x slower than DVE -- keep it DMA-only).  Phase B: the 5
            # combo products per row; combines read e2/e5 from SBUF so every
            # STT still has exactly one PSUM operand.
            es_d = tc.tile_pool(name="d", bufs=1, side="right")
            d_pool = es_d.__enter__()
            es_wo = tc.tile_pool(name="wo", bufs=4, side="right")
            wo_pool = es_wo.__enter__()
            es_osb = tc.tile_pool(name="osb", bufs=8, side="right")
            osb_pool = es_osb.__enter__()
            es_evA = tc.tile_pool(name="evA", bufs=1, side="right")
            evA_pool = es_evA.__enter__()
            d = {j: [] for j in (1, 3, 4, 6, 7)}
            d_specs = []
            for j, (qa, qb, op) in (
                (1, ("c11", "c22", ADD)), (4, ("c21", "c11", SUB)),
                (6, ("c11", "c12", ADD)), (3, ("c12", "c22", SUB)),
                (7, ("c21", "c22", ADD)),
            ):
                for kt in range(kt2):
                    d_specs.append((j, kt, qa, qb, op))
            dst = {"next": 0}

            def build_d(n):
                while dst["next"] < min(n, len(d_specs)):
                    j, kt, qa, qb, op = d_specs[dst["next"]]
                    t = d_pool.tile([P, NH], BF16, name=f"d{j}_{kt}",
                                    tag=f"d{j}_{kt}")
                    nc.vector.tensor_tensor(t[:], h2[qa][kt][:], h2[qb][kt][:], op)
                    d[j].append(t)
                    dst["next"] += 1

            oq = {"c11": 0, "c12": 1, "c21": 2, "c22": 3}
            ost = {"n": 0}

            def l3_out(quad, r, src):
                mo = oq[quad] * rt3 + r
                if r == rt3 - 1:
                    hw = NH // 2
                    nc.sync.dma_start(out=out_d[mo][:, 0:hw], in_=src[:, 0:hw])
                    nc.scalar.dma_start(out=out_d[mo][:, hw:], in_=src[:, hw:])
                else:
                    eng = nc.sync if ost["n"] % 2 else nc.scalar
                    eng.dma_start(out=out_d[mo], in_=src[:])
                    ost["n"] += 1

            orderA = ([(0, 2), (0, 5)] + [(r, 2) for r in range(1, rt3)]
                      + [(r, 5) for r in range(1, rt3)])
            orderB = [(r, j) for r in range(rt3) for j in (1, 4, 6, 3, 7)]
            order3 = orderA + orderB
            panels3 = {}
            pf3 = {"next": 0}

            def pump3(upto):
                while pf3["next"] <= min(upto, len(order3) - 1):
                    idx = pf3["next"]
                    r3, j3 = order3[idx]
                    t = wo_pool.tile([P, kt2 * P], BF16, tag="pan3")
                    rings[idx % 3].dma_start(out=t[:], in_=wo_d[j3][r3])
                    panels3[(r3, j3)] = t
                    pf3["next"] += 1

            def l3_mm(r, j, rhs_fn):
                pst = ps_pool.tile([P, NH], F32, tag="ps")
                pan = panels3.pop((r, j))
                for kt in range(kt2):
                    nc.tensor.matmul(
                        pst[:],
                        pan[:, kt * P:(kt + 1) * P],
                        rhs_fn(kt),
                        start=(kt == 0),
                        stop=(kt == kt2 - 1),
                    )
                return pst

            pump3(1)
            e25 = {}
            nc.scalar.dma_start(out=bot[:], in_=bot_d[:])
            nc.scalar.dma_start(out=bob[:], in_=bob_d[:])
            # interleave (r0,M2) with (r0,M5): their kt15 rhs tiles are L2's
            # last-row outputs
            pump3(1 + 2)
            pansA = [panels3.pop((0, 2)), panels3.pop((0, 5))]
            pstA = {}
            for j in (2, 5):
                pstA[j] = ps_pool.tile([P, NH], F32, name=f"psA{j}", tag="ps")
            for kt in range(kt2):
                for ji, j in enumerate((2, 5)):
                    srcq = h2["c11"] if j == 2 else h2["c22"]
                    nc.tensor.matmul(
                        pstA[j][:],
                        pansA[ji][:, kt * P:(kt + 1) * P],
                        srcq[kt][:],
                        start=(kt == 0),
                        stop=(kt == kt2 - 1),
                    )
            for j in (2, 5):
                ev = evA_pool.tile([P, NH], F32, name=f"eA{j}_0", tag=f"eA{j}_0")
                nc.scalar.activation(ev[:], pstA[j][:], IDENT)
                e25[(j, 0)] = ev
            build_d(20)
            for ai, (r, j) in enumerate(orderA):
                if ai < 2:
                    continue
                pump3(ai + 2)
                src = h2["c11"] if j == 2 else h2["c22"]
                pst = l3_mm(r, j, lambda kt: src[kt][:])
                ev = evA_pool.tile([P, NH], F32, name=f"eA{j}_{r}", tag=f"eA{j}_{r}")
                nc.scalar.activation(ev[:], pst[:], IDENT)
                e25[(j, r)] = ev
                build_d(10 * (ai + 1))
            build_d(len(d_specs))

            ps1 = c1 = c2 = d2 = None
            for bi, (r, j) in enumerate(orderB):
                pump3(len(orderA) + bi + 2)
                bt = bot[:, r:r + 1]
                bb = bob[:, r:r + 1]
                pst = l3_mm(r, j, lambda kt: d[j][kt][:])
                e2, e5 = e25[(2, r)], e25[(5, r)]
                if j == 1:
                    ps1 = pst
                    c1 = ch_pool.tile([P, NH], F32, tag="ch")
                    nc.vector.scalar_tensor_tensor(c1[:], pst[:], bt, e5[:], ADD, SUB)
                elif j == 4:
                    o21 = osb_pool.tile([P, NH], BF16, tag="osb")
                    nc.vector.scalar_tensor_tensor(o21[:], pst[:], bb, e2[:], ADD, ADD)
                    l3_out("c21", r, o21)
                    c2 = ch_pool.tile([P, NH], F32, tag="ch")
                    nc.vector.scalar_tensor_tensor(c2[:], pst[:], 1.0, c1[:], MULT, ADD)
                elif j == 6:
                    d1 = ch_pool.tile([P, NH], F32, tag="ch")
                    nc.vector.scalar_tensor_tensor(d1[:], pst[:], bb, e2[:], ADD, SUB)
                    d2 = ch_pool.tile([P, NH], F32, tag="ch")
                    nc.vector.scalar_tensor_tensor(d2[:], ps1[:], 1.0, d1[:], MULT, ADD)
                elif j == 3:
                    o12 = osb_pool.tile([P, NH], BF16, tag="osb")
                    nc.vector.scalar_tensor_tensor(o12[:], pst[:], bt, e5[:], ADD, ADD)
                    l3_out("c12", r, o12)
                    o22 = osb_pool.tile([P, NH], BF16, tag="osb")
                    nc.vector.scalar_tensor_tensor(o22[:], pst[:], 1.0, d2[:], MULT, ADD)
                    l3_out("c22", r, o22)
                elif j == 7:
                    o11 = osb_pool.tile([P, NH], BF16, tag="osb")
                    nc.vector.scalar_tensor_tensor(o11[:], pst[:], 1.0, c2[:], MULT, ADD)
                    l3_out("c11", r, o11)

            es_evA.__exit__(None, None, None)
            es_osb.__exit__(None, None, None)
            es_wo.__exit__(None, None, None)
            es_d.__exit__(None, None, None)
            es_h2c21.__exit__(None, None, None)
            es_h2c12.__exit__(None, None, None)
            es_h2c22.__exit__(None, None, None)
            es_h2c11.__exit__(None, None, None)

    nc.compile()
    return nc


def _expand_mask(mask, t=TILE):
    return np.repeat(np.repeat(np.asarray(mask, dtype=bool), t, axis=0), t, axis=1)


def _pack_lhsT(w):
    """[d_m, d_k] -> [d_m/P, P, d_k] lhsT panels (partition = contraction)."""
    d_m, d_k = w.shape
    mt, kt = d_m // P, d_k // P
    return np.ascontiguousarray(
        w.reshape(mt, P, kt, P).transpose(0, 3, 2, 1).reshape(mt, P, d_k)
    )


def _strassen_a(w):
    m, k = w.shape
    mh, kh = m // 2, k // 2
    A11, A12 = w[:mh, :kh], w[:mh, kh:]
    A21, A22 = w[mh:, :kh], w[mh:, kh:]
    return {
        1: A11 + A22, 2: A21 + A22, 3: A11, 4: A22,
        5: A11 + A12, 6: A21 - A11, 7: A12 - A22,
    }


def _pack_bias(b):
    n = b.shape[0] // P
    return np.ascontiguousarray(b.reshape(n, P).T)


def _run(x, w1e, b1, w2e, b2, wo, bo, d_in, d_h, d_out, n_cores=N_CORES, trace=False):
    b = x.shape[0]
    bc = b // n_cores
    assert bc == 2 * NH

    nc = bacc.Bacc("TRN2", target_bir_lowering=False, debug=False, num_devices=n_cores)
    _build(nc, d_in, d_h, d_out, bc)

    np_bf16 = mybir.dt.np(BF16)

    def cvt(a):
        return np.ascontiguousarray(a.astype(np_bf16))

    shared = {}
    for name, w in (("w1", w1e), ("w2", w2e), ("wo", wo)):
        for j, a in _strassen_a(w).items():
            shared[f"{name}_{j}"] = cvt(_pack_lhsT(a))
    shared["b1t"] = _pack_bias(b1[:d_h // 2])
    shared["b1b"] = _pack_bias(b1[d_h // 2:])
    shared["b2t"] = _pack_bias(b2[:d_h // 2])
    shared["b2b"] = _pack_bias(b2[d_h // 2:])
    shared["bot"] = _pack_bias(bo[:d_out // 2])
    shared["bob"] = _pack_bias(bo[d_out // 2:])

    kh1 = d_in // 2
    kt1 = kh1 // P
    in_maps = []
    for c in range(n_cores):
        xc_ = np.ascontiguousarray(x[c * bc:(c + 1) * bc].T)  # [d_in, bc]
        B11 = xc_[:kh1, :NH]
        B12 = xc_[:kh1, NH:]
        B21 = xc_[kh1:, :NH]
        B22 = xc_[kh1:, NH:]
        m = {
            "xb11": cvt(B11.reshape(kt1, P, NH)),
            "xb22": cvt(
                B22.reshape(2, 4, P, NH).transpose(0, 2, 1, 3).reshape(2, P, 4 * NH)
            ),
        }
        for j, comb in (
            (1, B11 + B22), (3, B12 - B22), (4, B21 - B11),
            (6, B11 + B12), (7, B21 + B22),
        ):
            m[f"xc{j}"] = cvt(
                comb.reshape(kt1, P, NH).transpose(1, 0, 2).reshape(P, kt1 * NH)
            )
        in_maps.append({**m, **shared})

    res = run_bass_kernel_spmd(nc, in_maps, core_ids=list(range(n_cores)), trace=trace)
    outs = []
    rt3 = d_out // 2 // P
    for c in range(n_cores):
        o = res.results[c]["out"].reshape(4, rt3 * P, NH).astype(np.float32)
        full = np.empty((d_out, bc), np.float32)
        full[:rt3 * P, :NH] = o[0]
        full[:rt3 * P, NH:] = o[1]
        full[rt3 * P:, :NH] = o[2]
        full[rt3 * P:, NH:] = o[3]
        outs.append(full)
    full = np.concatenate(outs, axis=1)  # [d_out, B]
    return np.ascontiguousarray(full.T), res


def kernel(x, W1, b1, W2, b2, Wo, bo, mask1, mask2):
    x = np.asarray(x, dtype=np.float32)
    w1e = np.asarray(W1, dtype=np.float32) * _expand_mask(mask1)
    w2e = np.asarray(W2, dtype=np.float32) * _expand_mask(mask2)
    out, _ = _run(
        x,
        w1e,
        np.asarray(b1, np.float32),
        w2e,
        np.asarray(b2, np.float32),
        np.asarray(Wo, np.float32),
        np.asarray(bo, np.float32),
        d_in=2048,
        d_h=4096,
        d_out=1024,
    )
    return out
